# revision 1
# baseline (speedup 1.0000x reference)
"""Causal GQA self-attention (B=1, T=2048, C=1024, 16 q-heads, 4 kv-groups, d=64)
on 8 Trainium2 NeuronCores.

Sharding: tensor-parallel over heads. Core c owns q-heads (2c, 2c+1) and kv-group
c//2. Each core computes x @ w_attn for its slice (transposed layout), RoPE,
causal flash-style attention for its 2 heads, and its partial y @ w_proj
(contracting only its 128 head-dims). Host sums the 8 partial outputs.

Pipeline (per core): x^T streams in as 4 column-chunks of 512; each chunk's
QKV projection + RoPE (stage1) feeds an attention window of 512 queries
(window w needs k-tiles 0..4w+3 only), whose epilogue projects and stores
directly from PSUM. Layout notes:
  - scores computed transposed sT[k, q]; softmax denominator via a ones
    column in the v operand; no max-subtraction (|scores| <= ~7).
  - vaug free layout [ones | v | ones] (66 cols): h0 uses cols 1:66 -> out
    partitions 0:65 (den at 64); h1 uses cols 0:65 -> out partitions 63:128
    (den at 63). No cross-partition moves needed to assemble y.
  - pt (exp output) and vaug are bf16: halves att@v operand traffic and
    avoids the 4x fp32r penalty on <256-free matmuls; accumulation stays f32.
  - causal mask on the diagonal 128x128 block added via a bf16 identity
    matmul of a -8e29 mask tile.
  - normalization: reciprocal of den row (DVE), PE broadcast to 64 rows,
    scale on DVE (h0) / Pool (h1); proj accumulates in PSUM and stores
    straight from PSUM to DRAM.
"""

import numpy as np
import ml_dtypes

import concourse.bass as bass
import concourse.mybir as mybir
from concourse import bacc
import concourse.tile as tile
from concourse.bass_utils import run_bass_kernel_spmd

T = 2048
C = 1024
D = 64
CW = 512                      # chunk / attention-window width
NW = T // CW                  # 4 windows
F32 = mybir.dt.float32
F32R = mybir.dt.float32r
BF16 = mybir.dt.bfloat16
EXP = mybir.ActivationFunctionType.Exp

_CACHE: dict = {}
PARTIAL = 3  # 1=stage1 only, 2=+attention, 3=full (for phase attribution)


def _rope_tables():
    # Replicate reference.apply_rope's f32 pipeline exactly.
    inv = (1.0 / (np.float32(10000.0) ** (np.arange(0, D, 2, dtype=np.float32) / np.float32(D)))).astype(np.float32)
    freqs = (np.arange(T, dtype=np.float32)[:, None] * inv[None, :]).astype(np.float32)  # (T, 32)
    freqs = np.repeat(freqs, 2, axis=1)                                                  # (T, 64)
    cos = np.cos(freqs).astype(np.float32).T.copy()                                      # (64, T)
    sin = np.sin(freqs).astype(np.float32).T.copy()
    cos2 = np.ascontiguousarray(np.concatenate([cos, cos], axis=0))                      # (128, T)
    sin2 = np.ascontiguousarray(np.concatenate([sin, sin], axis=0))
    return cos2, sin2


def _const_mats():
    # perm (as lhsT): rot[2i] = -x[2i+1], rot[2i+1] = +x[2i]
    perm = np.zeros((128, 128), np.float32)
    for i in range(64):
        perm[2 * i + 1, 2 * i] = -1.0
        perm[2 * i, 2 * i + 1] = 1.0
    ident = np.eye(128, dtype=np.float32)
    shift = np.zeros((128, 128), np.float32)   # [64+i, i] = 1 down-shift; [i, 64+i] = 1 up-shift
    for i in range(64):
        shift[64 + i, i] = 1.0
        shift[i, 64 + i] = 1.0
    kq = np.arange(128)
    binmask = np.where(kq[:, None] <= kq[None, :], 0.0, -8e29).astype(np.float32)  # [k, q] additive
    return perm, ident, shift, binmask


def _build_bass(repeat: int = 1) -> bass.Bass:
    nc = bacc.Bacc(None, target_bir_lowering=False)
    xt_d = nc.dram_tensor("xt", [C, T], BF16, kind="ExternalInput")
    wqkv = nc.dram_tensor("wqkv", [C, 4 * D], BF16, kind="ExternalInput")
    wproj = nc.dram_tensor("wproj", [2 * D, C], BF16, kind="ExternalInput")
    out = nc.dram_tensor("out", [T, C], BF16, kind="ExternalOutput")

    cos2_np, sin2_np = _rope_tables()
    perm_np, ident_np, shift_np, binmask_np = _const_mats()
    cmat_np = np.ascontiguousarray(
        np.concatenate([perm_np, ident_np, shift_np, binmask_np], axis=1)
    )  # [128, 512]: perm | ident | shift | binmask
    cos_d = nc.inline_tensor(cos2_np, name="cos2")
    sin_d = nc.inline_tensor(sin2_np, name="sin2")
    cmat_d = nc.inline_tensor(cmat_np, name="cmatm")

    with tile.TileContext(nc) as tc:
        with (
            nc.allow_low_precision(reason="fp32r/bf16 rounding of matmul operands"),
            tc.tile_pool(name="const", bufs=1) as const,
            tc.tile_pool(name="big", bufs=1) as big,
            tc.tile_pool(name="work", bufs=1) as work,
            tc.tile_pool(name="ps", bufs=1, space="PSUM") as psp,
        ):
            SB = 3  # shared 1-bank psum slots

            # Everything rides the SP HWDGE queue in dependency-need order:
            # weights, chunk 0, constants, then later chunks interleaved with
            # their cos/sin slices (transfer order == issue order, so early
            # consumers aren't stuck behind 8MB of x^T).
            w_r = const.tile([128, 8, 4 * D], BF16)
            wqkv_src = wqkv.rearrange("(c p) n -> p c n", p=128)
            nc.sync.dma_start(out=w_r[:, :, 0:128], in_=wqkv_src[:, :, 0:128])
            xt = big.tile([128, 8, T], BF16)  # x^T as 8 c-tiles (host-transposed input)
            xt_src = xt_d.rearrange("(c p) t -> p c t", p=128)

            def load_xt_chunk(w):
                sl = slice(CW * w, CW * (w + 1))
                nc.sync.dma_start(out=xt[:, :, sl], in_=xt_src[:, :, sl])

            load_xt_chunk(0)
            nc.sync.dma_start(out=w_r[:, :, 128:256], in_=wqkv_src[:, :, 128:256])

            cmat_f = const.tile([128, 512], F32)
            nc.sync.dma_start(out=cmat_f, in_=cmat_d[:, :])
            id_sb = cmat_f[:, 128:256]
            cos_sb = const.tile([128, T], F32)
            sin_sb = const.tile([128, T], F32)

            def load_trig_chunk(w):
                sl = slice(CW * w, CW * (w + 1))
                nc.sync.dma_start(out=cos_sb[:, sl], in_=cos_d[:, sl])
                nc.sync.dma_start(out=sin_sb[:, sl], in_=sin_d[:, sl])

            load_trig_chunk(0)
            # w_proj halves, both at partitions 0:64 so per-head proj matmuls
            # avoid cross-partition moves of y
            wpc0 = const.tile([64, C], BF16)
            nc.sync.dma_start(out=wpc0, in_=wproj[0:64, :])
            wpc1 = const.tile([64, C], BF16)
            nc.sync.dma_start(out=wpc1, in_=wproj[64:128, :])

            perm_sb = const.tile([128, 128], F32R)
            nc.vector.tensor_copy(out=perm_sb, in_=cmat_f[:, 0:128])
            sh_sb = const.tile([128, 128], BF16)
            nc.vector.tensor_copy(out=sh_sb, in_=cmat_f[:, 256:384])
            idb_sb = const.tile([128, 128], BF16)
            nc.vector.tensor_copy(out=idb_sb, in_=cmat_f[:, 128:256])
            mkb_sb = const.tile([128, 128], BF16)
            nc.vector.tensor_copy(out=mkb_sb, in_=cmat_f[:, 384:512])
            ones_f = const.tile([128, 64], F32)
            nc.vector.memset(ones_f, 1.0)
            ones_sb = const.tile([65, 64], F32R)
            nc.vector.tensor_copy(out=ones_sb, in_=ones_f[0:65, :])

            for _w in range(1, NW):
                load_xt_chunk(_w)
                load_trig_chunk(_w)

            # persistent per-core activations
            qrope_sb = big.tile([128, T], BF16)   # roped q, h0 rows 0:64, h1 rows 64:128
            q1_sb = big.tile([64, T], BF16)       # roped q of h1 shifted to partitions 0:64
            krope_sb = big.tile([64, T], BF16)
            # v tiles in [t, d] layout, bf16, free cols: [v(64) | ones]
            vaug_sb = big.tile([128, 16, 65], BF16)
            nc.vector.memset(vaug_sb[:, :, 64:65], 1.0)

            # ------- stage 1: QKV projection + RoPE for one 512-chunk of T.
            # Generator: yields between pieces so the driver can weave these
            # instructions between the previous window's attention steps.
            def stage1_units(r, w):
                sl = slice(CW * w, CW * (w + 1))
                qraw = work.tile([128, CW], F32R, tag="qraw", bufs=2, name=f"x{r}qraw{w}")
                kvraw = work.tile([128, CW], F32R, tag="kvraw", bufs=2, name=f"x{r}kvraw{w}")
                for m, dst in ((0, qraw), (1, kvraw)):
                    ps = psp.tile([128, CW], F32, tag="s", bufs=SB, name=f"x{r}qkv{w}_{m}")
                    for c in range(8):
                        nc.tensor.matmul(
                            ps,
                            lhsT=w_r[:, c, 128 * m : 128 * (m + 1)],
                            rhs=xt[:, c, sl],
                            start=(c == 0),
                            stop=(c == 7),
                        )
                        if c == 3:
                            yield
                    nc.vector.tensor_copy(out=dst, in_=ps)
                    yield
                tmp = work.tile([128, CW], F32, tag="tmp", bufs=2, name=f"x{r}tmp{w}")
                tmpq = work.tile([128, CW], F32, tag="tmpq", bufs=2, name=f"x{r}tmpq{w}")
                tmpk = work.tile([64, CW], F32, tag="tmpk", bufs=2, name=f"x{r}tmpk{w}")
                tmpk2 = work.tile([64, CW], F32, tag="tmpk2", bufs=2, name=f"x{r}tmpk2{w}")
                # rope q (both heads at once); final add converts to bf16
                rps = psp.tile([128, CW], F32, tag="s", bufs=SB, name=f"x{r}rot{w}")
                nc.tensor.matmul(rps, lhsT=perm_sb, rhs=qraw, start=True, stop=True)
                nc.vector.tensor_mul(tmp, rps, sin_sb[:, sl])
                nc.gpsimd.tensor_mul(tmpq, qraw, cos_sb[:, sl])
                nc.gpsimd.tensor_add(qrope_sb[:, sl], tmpq, tmp)
                yield
                # rope k (rows 0:64 of kvraw)
                rpsk = psp.tile([64, CW], F32, tag="s", bufs=SB, name=f"x{r}rotk{w}")
                nc.tensor.matmul(rpsk, lhsT=perm_sb[0:64, 0:64], rhs=kvraw[0:64, :], start=True, stop=True)
                nc.vector.tensor_mul(tmpk, rpsk, sin_sb[0:64, sl])
                nc.gpsimd.tensor_mul(tmpk2, kvraw[0:64, :], cos_sb[0:64, sl])
                nc.gpsimd.tensor_add(krope_sb[:, sl], tmpk2, tmpk)
                yield
                # shift roped h1 q down to partitions 0:64
                sps = psp.tile([64, CW], F32, tag="s", bufs=SB, name=f"x{r}shift{w}")
                nc.tensor.matmul(sps, lhsT=sh_sb[64:128, 0:64], rhs=qrope_sb[64:128, sl], start=True, stop=True)
                nc.vector.tensor_copy(out=q1_sb[:, sl], in_=sps)
                yield
                # v_aug: transpose v tiles of this chunk into [t, d] bf16
                for tt in range(4 * w, 4 * w + 4):
                    vps = psp.tile([128, 64], F32, tag="s", bufs=SB, name=f"x{r}vtr{tt}")
                    nc.tensor.transpose(
                        vps,
                        in_=kvraw[64:128, 128 * (tt - 4 * w) : 128 * (tt - 4 * w + 1)].bitcast(F32),
                        identity=id_sb[64:128, 64:128],
                    )
                    nc.vector.tensor_copy(out=vaug_sb[:, tt, 0:64], in_=vps)
                    if tt % 2 == 1:
                        yield

            # ------- attention window of CW queries (both heads); yields per
            # k-tile step so stage1(w+1) / epilogue(w-1) can interleave.
            def attn_units(r, w, out_yps):
                ktiles = 4 * w + 4
                yps = {
                    h: psp.tile([65, CW], F32, tag=f"yt{h}", bufs=2, name=f"x{r}yps{w}_{h}")
                    for h in range(2)
                }
                out_yps.update(yps)

                def emit_yt(jj, ptsj):
                    gg = jj - 4 * w
                    qq0 = max(gg, 0) * 128
                    for h in range(2):
                        nc.tensor.matmul(
                            yps[h][:, qq0:CW],
                            lhsT=vaug_sb[:, jj, :],
                            rhs=ptsj[h][:, 0 : CW - qq0],
                            start=(jj == 0),
                            stop=(jj == ktiles - 1),
                            skip_group_check=True,
                        )

                prev = None
                for j in range(ktiles):
                    g = j - 4 * w
                    q0 = max(g, 0) * 128
                    pts = {}
                    for h in range(2):
                        qsrc = qrope_sb if h == 0 else q1_sb
                        pt = work.tile([128, CW], BF16, tag="pt", bufs=6, name=f"x{r}pt{w}_{j}_{h}")
                        spsm = psp.tile([128, CW], F32, tag="s", bufs=SB, name=f"x{r}s{w}_{j}_{h}")
                        nc.tensor.matmul(
                            spsm[:, 0 : CW - q0],
                            lhsT=krope_sb[:, 128 * j : 128 * (j + 1)],
                            rhs=qsrc[0:64, CW * w + q0 : CW * (w + 1)],
                            start=True,
                            stop=(g < 0),
                            skip_group_check=True,
                        )
                        if g >= 0:
                            nc.tensor.matmul(
                                spsm[:, 0:128],
                                lhsT=idb_sb,
                                rhs=mkb_sb,
                                start=False,
                                stop=True,
                                skip_group_check=True,
                            )
                        nc.scalar.activation(
                            out=pt[:, 0 : CW - q0], in_=spsm[:, 0 : CW - q0], func=EXP, scale=0.125
                        )
                        pts[h] = pt
                    if prev is not None:
                        emit_yt(*prev)
                    prev = (j, pts)
                    yield
                emit_yt(*prev)

            # ------- epilogue: normalize, project, store; yields between
            # pieces so it can hide under the next window's attention.
            def epilogue_units(r, w, yps):
                r0 = work.tile([65, CW], F32R, tag="r0", bufs=2, name=f"x{r}r0_{w}")
                r1 = work.tile([65, CW], F32R, tag="r1", bufs=2, name=f"x{r}r1_{w}")
                nc.vector.reciprocal(out=r0[64:65, :], in_=yps[0][64:65, :])
                nc.vector.reciprocal(out=r1[64:65, :], in_=yps[1][64:65, :])
                rbps0 = psp.tile([64, CW], F32, tag="s", bufs=SB, name=f"x{r}rb0_{w}")
                nc.tensor.matmul(
                    rbps0, lhsT=ones_sb[64:65, 0:64], rhs=r0[64:65, :],
                    start=True, stop=True, skip_group_check=True,
                )
                rbps1 = psp.tile([64, CW], F32, tag="s", bufs=SB, name=f"x{r}rb1_{w}")
                nc.tensor.matmul(
                    rbps1, lhsT=ones_sb[64:65, 0:64], rhs=r1[64:65, :],
                    start=True, stop=True, skip_group_check=True,
                )
                rb0_sb = work.tile([64, CW], F32, tag="rb0", bufs=2, name=f"x{r}rbs0_{w}")
                nc.vector.tensor_copy(out=rb0_sb, in_=rbps0)
                rb1_sb = work.tile([64, CW], F32, tag="rb1", bufs=2, name=f"x{r}rbs1_{w}")
                nc.vector.tensor_copy(out=rb1_sb, in_=rbps1)
                yield
                yn0 = work.tile([64, CW], BF16, tag="yn0", bufs=2, name=f"x{r}yn0_{w}")
                yn1 = work.tile([64, CW], BF16, tag="yn1", bufs=2, name=f"x{r}yn1_{w}")
                nc.vector.tensor_mul(yn0, yps[0][0:64, :], rb0_sb)
                nc.vector.tensor_mul(yn1, yps[1][0:64, :], rb1_sb)
                yield
                for t4 in range(4):
                    tglob = 4 * w + t4
                    tsl = slice(128 * t4, 128 * (t4 + 1))
                    osb = work.tile([128, C], BF16, tag="o", bufs=3, name=f"x{r}o{w}_{t4}")
                    for n2 in range(2):
                        nsl = slice(512 * n2, 512 * (n2 + 1))
                        ops_ = psp.tile([128, 512], F32, tag="o", bufs=1, name=f"x{r}op{w}_{t4}_{n2}")
                        nc.tensor.matmul(
                            ops_, lhsT=yn0[:, tsl], rhs=wpc0[:, nsl],
                            start=True, stop=False, skip_group_check=True,
                        )
                        nc.tensor.matmul(
                            ops_, lhsT=yn1[:, tsl], rhs=wpc1[:, nsl],
                            start=False, stop=True, skip_group_check=True,
                        )
                        nc.vector.tensor_copy(out=osb[:, nsl], in_=ops_)
                    nc.sync.dma_start(out=out[128 * tglob : 128 * (tglob + 1), :], in_=osb)
                    yield

            def drain(gen, n=None):
                if gen is None:
                    return None
                try:
                    if n is None:
                        for _ in gen:
                            pass
                        return None
                    for _ in range(n):
                        next(gen)
                    return gen
                except StopIteration:
                    return None

            # ------- anchor stores so partial builds aren't dead-code -------
            def anchor_stage1(r):
                asb = work.tile([128, C], BF16, tag="o", bufs=3, name=f"anch{r}")
                nc.vector.tensor_copy(out=asb[:, 0:512], in_=qrope_sb[:, 0:512])
                nc.vector.tensor_copy(out=asb[0:64, 512:1024], in_=krope_sb[:, 0:512])
                nc.vector.tensor_copy(out=asb[64:128, 512:768], in_=q1_sb[0:64, 0:256])
                for _vt in range(4):
                    nc.vector.tensor_copy(out=asb[64:128, 768 + 64 * _vt : 832 + 64 * _vt], in_=vaug_sb[64:128, _vt, 0:64])
                nc.sync.dma_start(out=out[128 * (r % 16) : 128 * (r % 16 + 1), :], in_=asb)

            def anchor_attn(r, w, yps):
                asb = work.tile([128, C], BF16, tag="o", bufs=3, name=f"anch{r}_{w}")
                nc.vector.tensor_copy(out=asb[0:65, 0:512], in_=yps[0])
                nc.vector.tensor_copy(out=asb[0:65, 512:1024], in_=yps[1])
                nc.sync.dma_start(out=out[128 * (4 * (r % 4) + w) : 128 * (4 * (r % 4) + w) + 128, :], in_=asb)
                if False:
                    yield

            if PARTIAL == 1:
                for r in range(repeat):
                    for w in range(NW):
                        if w == NW - 1 and r + 1 < repeat:
                            for cw in range(NW):
                                load_xt_chunk(cw)
                        drain(stage1_units(r, w))
                    anchor_stage1(r)
                repeat = 0  # skip the full drive below

            # ------- software-pipelined drive over (rep, window) -------
            if repeat:
                first = stage1_units(0, 0)
                drain(first)
            epi_pend = None
            for r in range(repeat):
                for w in range(NW):
                    if w == NW - 1 and r + 1 < repeat:
                        # next rep's x^T reloads; transfers overlap this window
                        for cw in range(NW):
                            load_xt_chunk(cw)
                    if w + 1 < NW:
                        nxt = stage1_units(r, w + 1)
                    elif r + 1 < repeat:
                        nxt = stage1_units(r + 1, 0)
                    else:
                        nxt = None
                    yps = {}
                    for _ in attn_units(r, w, yps):
                        nxt = drain(nxt, 3)
                        epi_pend = drain(epi_pend, 1)
                    drain(nxt)
                    drain(epi_pend)
                    if PARTIAL == 2:
                        epi_pend = anchor_attn(r, w, yps)
                    else:
                        epi_pend = epilogue_units(r, w, yps)
            drain(epi_pend)
    nc.finalize()
    return nc


def _get_nc(repeat: int = 1) -> bass.Bass:
    key = ("nc", repeat)
    if key not in _CACHE:
        _CACHE[key] = _build_bass(repeat)
    return _CACHE[key]


def _make_in_maps(x, w_attn, w_proj):
    x2 = np.ascontiguousarray(np.asarray(x, dtype=np.float32).reshape(T, C).T).astype(ml_dtypes.bfloat16)  # [C, T]
    wr = np.asarray(w_attn, dtype=np.float32).reshape(C, 4, 6, D).astype(ml_dtypes.bfloat16)
    wp = np.asarray(w_proj, dtype=np.float32).astype(ml_dtypes.bfloat16)
    in_maps = []
    for c in range(8):
        g = c // 2
        s = (2 * c) % 4
        wqkv_c = np.ascontiguousarray(
            np.concatenate([wr[:, g, s, :], wr[:, g, s + 1, :], wr[:, g, 4, :], wr[:, g, 5, :]], axis=1)
        )
        wproj_c = np.ascontiguousarray(wp[128 * c : 128 * (c + 1), :])
        in_maps.append({"xt": x2, "wqkv": wqkv_c, "wproj": wproj_c})
    return in_maps


def _combine(results):
    acc = np.zeros((T, C), np.float64)
    for r in results:
        acc += r["out"]
    return acc.astype(np.float32).reshape(1, T, C)


def run_for_test(inputs, trace=False):
    """Returns (output, exec_time_ns_or_None). Used by test.py."""
    nc = _get_nc()
    in_maps = _make_in_maps(**inputs)
    res = run_bass_kernel_spmd(nc, in_maps, core_ids=list(range(8)), trace=trace)
    return _combine(res.results), res.exec_time_ns


def kernel(x, w_attn, w_proj):
    out, _ = run_for_test({"x": x, "w_attn": w_attn, "w_proj": w_proj})
    return out



# revision 3
# speedup vs baseline: 1.4122x; 1.4122x over previous
"""Causal GQA self-attention (B=1, T=2048, C=1024, 16 q-heads, 4 kv-groups, d=64)
on 8 Trainium2 NeuronCores — v2.

Sharding: tensor-parallel over heads. Core c owns q-heads (2c, 2c+1) and kv-group
c//2. Each core computes x @ w_attn for its slice (transposed layout), RoPE,
causal flash-style attention for its 2 heads, and its partial y @ w_proj
(contracting its 128 head-dims in one 128-deep matmul). Host sums the partials.

v2 changes vs baseline (157 -> ~96-100us per-rep slope):
  - both heads' scores live in one [128, 2, CW] psum per k-step, so each step
    runs ONE combined exp on ACT (half the ACT ops and cross-engine sem hops).
  - merged 128-contract proj: yn holds h0 at partitions 0:64 (DVE mul) and h1
    at 64:128 (shifted-out-base DVE mul), halving the proj matmul count.
  - output stores batched one [128, 4096] tile per window with the DRAM layout
    matching SBUF exactly (host unpermutes in _combine): 128 contiguous 8KB
    descriptors per store, issued on the scalar HWDGE ring so they never
    head-of-line-block the xt loads on the SP ring.
  - host pre-tiles xt -> [128, w, c, t'] and wqkv -> [128, c, n] so every load
    is >=4KB-contiguous per partition; trig rides one inline tensor.
  - xt reloads for the next rep spread over windows 2-3.
  - engine balance respects HW rules: GPSIMD never touches PSUM (rope SBUF
    muls/adds only); PSUM reads go to DVE; matmul psum bases at 0/64 only and
    never across a 2KB bank.
"""

import numpy as np
import ml_dtypes

import concourse.bass as bass
import concourse.mybir as mybir
from concourse import bacc
import concourse.tile as tile
from concourse.bass_utils import run_bass_kernel_spmd

T = 2048
C = 1024
D = 64
CW = 512                      # chunk / attention-window width
NW = T // CW                  # 4 windows
F32 = mybir.dt.float32
F32R = mybir.dt.float32r
BF16 = mybir.dt.bfloat16
EXP = mybir.ActivationFunctionType.Exp

_CACHE: dict = {}
PARTIAL = 3  # 1=stage1 only, 2=+attention, 3=full (phase attribution)
EPI_FIRST = True   # drain epilogue before stage1 in the weave, 2 units/step
OSB_ON_ACT = False  # n2==1 osb copies on ACT (else DVE)


def _rope_tables():
    inv = (1.0 / (np.float32(10000.0) ** (np.arange(0, D, 2, dtype=np.float32) / np.float32(D)))).astype(np.float32)
    freqs = (np.arange(T, dtype=np.float32)[:, None] * inv[None, :]).astype(np.float32)  # (T, 32)
    freqs = np.repeat(freqs, 2, axis=1)                                                  # (T, 64)
    cos = np.cos(freqs).astype(np.float32).T.copy()                                      # (64, T)
    sin = np.sin(freqs).astype(np.float32).T.copy()
    cos2 = np.concatenate([cos, cos], axis=0)                                            # (128, T)
    sin2 = np.concatenate([sin, sin], axis=0)
    return np.ascontiguousarray(np.concatenate([cos2, sin2], axis=1))                    # (128, 2T)


def _const_mats():
    # perm (as lhsT): rot[2i] = -x[2i+1], rot[2i+1] = +x[2i]
    perm = np.zeros((128, 128), np.float32)
    for i in range(64):
        perm[2 * i + 1, 2 * i] = -1.0
        perm[2 * i, 2 * i + 1] = 1.0
    ident = np.eye(128, dtype=np.float32)
    shift = np.zeros((128, 128), np.float32)   # [64+i, i] = 1 down-shift; [i, 64+i] = 1 up-shift
    for i in range(64):
        shift[64 + i, i] = 1.0
        shift[i, 64 + i] = 1.0
    kq = np.arange(128)
    binmask = np.where(kq[:, None] <= kq[None, :], 0.0, -8e29).astype(np.float32)  # [k, q] additive
    return np.ascontiguousarray(np.concatenate([perm, ident, shift, binmask], axis=1))


def _build_bass(repeat: int = 1) -> bass.Bass:
    nc = bacc.Bacc(None, target_bir_lowering=False)
    xt_d = nc.dram_tensor("xt", [128, NW * 8 * CW], BF16, kind="ExternalInput")
    wqkv = nc.dram_tensor("wqkv", [128, 8 * 256], BF16, kind="ExternalInput")
    wproj = nc.dram_tensor("wproj", [128, C], BF16, kind="ExternalInput")
    # out stored window-major as [w*128 + p, t4*C + c] so each window's store
    # is one 8KB-contiguous run per partition; host unpermutes in _combine.
    out = nc.dram_tensor("out", [NW * 128, 4 * C], BF16, kind="ExternalOutput")

    trig_d = nc.inline_tensor(_rope_tables(), name="trig2")
    cmat_d = nc.inline_tensor(_const_mats(), name="cmatm")

    xt_src = xt_d.rearrange("p (w c t) -> p w c t", w=NW, c=8, t=CW)

    with tile.TileContext(nc) as tc:
        with (
            nc.allow_low_precision(reason="fp32r/bf16 rounding of matmul operands"),
            tc.tile_pool(name="const", bufs=1) as const,
            tc.tile_pool(name="big", bufs=1) as big,
            tc.tile_pool(name="work", bufs=1) as work,
            tc.tile_pool(name="ps", bufs=1, space="PSUM") as psp,
        ):
            SB = 2  # shared 1-bank psum slots (stage1 + bcast + proj)

            # loads in dependency-need order on the SP ring
            w_r = const.tile([128, 8, 256], BF16)
            nc.sync.dma_start(out=w_r, in_=wqkv.rearrange("p (c n) -> p c n", c=8))
            xt = big.tile([128, NW, 8, CW], BF16)

            def load_xt_chunk(w):
                nc.sync.dma_start(out=xt[:, w], in_=xt_src[:, w])

            load_xt_chunk(0)
            cmat_f = const.tile([128, 512], F32)
            nc.sync.dma_start(out=cmat_f, in_=cmat_d[:, :])
            id_sb = cmat_f[:, 128:256]
            trig_sb = const.tile([128, 2, T], F32)
            nc.sync.dma_start(out=trig_sb[:, :, 0:CW], in_=trig_d.rearrange("p (k t) -> p k t", k=2)[:, :, 0:CW])
            wpf = const.tile([128, C], BF16)
            nc.sync.dma_start(out=wpf, in_=wproj[:, :])
            nc.sync.dma_start(out=trig_sb[:, :, CW:T], in_=trig_d.rearrange("p (k t) -> p k t", k=2)[:, :, CW:T])
            cos_sb = trig_sb[:, 0, :]
            sin_sb = trig_sb[:, 1, :]

            perm_sb = const.tile([128, 128], F32R)
            nc.vector.tensor_copy(out=perm_sb, in_=cmat_f[:, 0:128])
            sh_sb = const.tile([128, 128], BF16)
            nc.vector.tensor_copy(out=sh_sb, in_=cmat_f[:, 256:384])
            idb_sb = const.tile([128, 128], BF16)
            nc.vector.tensor_copy(out=idb_sb, in_=cmat_f[:, 128:256])
            mkb_sb = const.tile([128, 128], BF16)
            nc.vector.tensor_copy(out=mkb_sb, in_=cmat_f[:, 384:512])
            ones_f = const.tile([128, 64], F32)
            nc.vector.memset(ones_f, 1.0)
            ones_sb = const.tile([65, 64], F32R)
            nc.vector.tensor_copy(out=ones_sb, in_=ones_f[0:65, :])

            for _w in range(1, NW):
                load_xt_chunk(_w)

            # persistent per-core activations
            qrope_sb = big.tile([128, T], BF16)   # roped q, h0 rows 0:64, h1 rows 64:128
            q1_sb = big.tile([64, T], BF16)       # roped q of h1 shifted to partitions 0:64
            krope_sb = big.tile([64, T], BF16)
            # v tiles in [t, d] layout, bf16, free cols: [v | ones]
            vaug_sb = big.tile([128, 16, 65], BF16)
            nc.vector.memset(vaug_sb[:, :, 64:65], 1.0)

            # ------- stage 1: QKV projection + RoPE for one 512-chunk of T.
            def stage1_units(r, w):
                sl = slice(CW * w, CW * (w + 1))
                qraw = work.tile([128, CW], F32R, tag="qraw", bufs=2, name=f"x{r}qraw{w}")
                kvraw = work.tile([128, CW], F32R, tag="kvraw", bufs=2, name=f"x{r}kvraw{w}")
                for m, dst in ((0, qraw), (1, kvraw)):
                    ps = psp.tile([128, CW], F32, tag="s", bufs=SB, name=f"x{r}qkv{w}_{m}")
                    for c in range(8):
                        nc.tensor.matmul(
                            ps,
                            lhsT=w_r[:, c, 128 * m : 128 * (m + 1)],
                            rhs=xt[:, w, c, :],
                            start=(c == 0),
                            stop=(c == 7),
                        )
                        if c == 3:
                            yield
                    nc.vector.tensor_copy(out=dst, in_=ps)
                    yield
                tmp = work.tile([128, CW], F32, tag="tmp", bufs=2, name=f"x{r}tmp{w}")
                tmpq = work.tile([128, CW], F32, tag="tmpq", bufs=2, name=f"x{r}tmpq{w}")
                tmpk = work.tile([64, CW], F32, tag="tmpk", bufs=2, name=f"x{r}tmpk{w}")
                tmpk2 = work.tile([64, CW], F32, tag="tmpk2", bufs=2, name=f"x{r}tmpk2{w}")
                # rope q (both heads at once); final add converts to bf16
                rps = psp.tile([128, CW], F32, tag="s", bufs=SB, name=f"x{r}rot{w}")
                nc.tensor.matmul(rps, lhsT=perm_sb, rhs=qraw, start=True, stop=True)
                nc.vector.tensor_mul(tmp, rps, sin_sb[:, sl])
                nc.gpsimd.tensor_mul(tmpq, qraw, cos_sb[:, sl])
                nc.gpsimd.tensor_add(qrope_sb[:, sl], tmpq, tmp)
                yield
                # rope k (rows 0:64 of kvraw)
                rpsk = psp.tile([64, CW], F32, tag="s", bufs=SB, name=f"x{r}rotk{w}")
                nc.tensor.matmul(rpsk, lhsT=perm_sb[0:64, 0:64], rhs=kvraw[0:64, :], start=True, stop=True)
                nc.vector.tensor_mul(tmpk, rpsk, sin_sb[0:64, sl])
                nc.gpsimd.tensor_mul(tmpk2, kvraw[0:64, :], cos_sb[0:64, sl])
                nc.gpsimd.tensor_add(krope_sb[:, sl], tmpk2, tmpk)
                yield
                # shift roped h1 q down to partitions 0:64
                sps = psp.tile([64, CW], F32, tag="s", bufs=SB, name=f"x{r}shift{w}")
                nc.tensor.matmul(sps, lhsT=sh_sb[64:128, 0:64], rhs=qrope_sb[64:128, sl], start=True, stop=True)
                nc.vector.tensor_copy(out=q1_sb[:, sl], in_=sps)
                yield
                # v_aug: transpose v tiles of this chunk into [t, d] bf16
                for tt in range(4 * w, 4 * w + 4):
                    vps = psp.tile([128, 64], F32, tag="s", bufs=SB, name=f"x{r}vtr{tt}")
                    nc.tensor.transpose(
                        vps,
                        in_=kvraw[64:128, 128 * (tt - 4 * w) : 128 * (tt - 4 * w + 1)].bitcast(F32),
                        identity=id_sb[64:128, 64:128],
                    )
                    nc.vector.tensor_copy(out=vaug_sb[:, tt, 0:64], in_=vps)
                    if tt % 2 == 1:
                        yield

            # ------- attention window of CW queries; both heads share one
            # [128, 2, CW] score psum so each j-step has ONE exp and ONE av
            # matmul (fewer ACT ops and cross-engine sem hops). Yields per
            # k-tile step so stage1(w+1) / epilogue(w-1) can interleave.
            def attn_units(r, w, out_yps):
                ktiles = 4 * w + 4
                ypsc = psp.tile([65, 2, CW], F32, tag="yt", bufs=1, name=f"x{r}yps{w}")
                out_yps[0] = ypsc

                def emit_yt(jj, ptj):
                    gg = jj - 4 * w
                    qq0 = max(gg, 0) * 128
                    # one matmul per head: a psum matmul target cannot cross
                    # its 2KB bank boundary (shared vaug lhsT -> one LDW)
                    for h in range(2):
                        nc.tensor.matmul(
                            ypsc[:, h, qq0:CW],
                            lhsT=vaug_sb[:, jj, :],
                            rhs=ptj[:, h, 0 : CW - qq0],
                            start=(jj == 0),
                            stop=(jj == ktiles - 1),
                            skip_group_check=True,
                        )

                prev = None
                for j in range(ktiles):
                    g = j - 4 * w
                    q0 = max(g, 0) * 128
                    pt = work.tile([128, 2, CW], BF16, tag="pt", bufs=4, name=f"x{r}pt{w}_{j}")
                    spsm = psp.tile([128, 2, CW], F32, tag="satt", bufs=2, name=f"x{r}s{w}_{j}")
                    for h in range(2):
                        qsrc = qrope_sb if h == 0 else q1_sb
                        nc.tensor.matmul(
                            spsm[:, h, 0 : CW - q0],
                            lhsT=krope_sb[:, 128 * j : 128 * (j + 1)],
                            rhs=qsrc[0:64, CW * w + q0 : CW * (w + 1)],
                            start=True,
                            stop=(g < 0),
                            skip_group_check=True,
                        )
                    if g >= 0:
                        for h in range(2):
                            nc.tensor.matmul(
                                spsm[:, h, 0:128],
                                lhsT=idb_sb,
                                rhs=mkb_sb,
                                start=False,
                                stop=True,
                                skip_group_check=True,
                            )
                    nc.scalar.activation(
                        out=pt[:, :, 0 : CW - q0], in_=spsm[:, :, 0 : CW - q0], func=EXP, scale=0.125
                    )
                    if prev is not None:
                        emit_yt(*prev)
                    prev = (j, pt)
                    yield
                emit_yt(*prev)

            # ------- epilogue: normalize, project (128-contract), store.
            def epilogue_units(r, w, yps):
                ypsc = yps[0]
                rc = work.tile([65, 2, CW], F32R, tag="rc", bufs=2, name=f"x{r}rc_{w}")
                nc.vector.reciprocal(out=rc[64:65, :, :], in_=ypsc[64:65, :, :])
                yield
                rbps0 = psp.tile([64, CW], F32, tag="s", bufs=SB, name=f"x{r}rb0_{w}")
                nc.tensor.matmul(
                    rbps0, lhsT=ones_sb[64:65, 0:64], rhs=rc[64:65, 0, :],
                    start=True, stop=True, skip_group_check=True,
                )
                rbps1 = psp.tile([64, CW], F32, tag="s", bufs=SB, name=f"x{r}rb1_{w}")
                nc.tensor.matmul(
                    rbps1, lhsT=ones_sb[64:65, 0:64], rhs=rc[64:65, 1, :],
                    start=True, stop=True, skip_group_check=True,
                )
                rb0_sb = work.tile([64, CW], F32, tag="rb0", bufs=2, name=f"x{r}rbs0_{w}")
                nc.vector.tensor_copy(out=rb0_sb, in_=rbps0)
                rb1_sb = work.tile([64, CW], F32, tag="rb1", bufs=2, name=f"x{r}rbs1_{w}")
                nc.vector.tensor_copy(out=rb1_sb, in_=rbps1)
                yield
                # merged-head yn: h0 -> partitions 0:64 (DVE), h1 -> 64:128
                # (Pool, shifted out base)
                yn = work.tile([128, CW], BF16, tag="yn", bufs=2, name=f"x{r}yn_{w}")
                nc.vector.tensor_mul(yn[0:64, :], ypsc[0:64, 0, :], rb0_sb)
                nc.vector.tensor_mul(yn[64:128, :], ypsc[0:64, 1, :], rb1_sb)
                yield
                osb = work.tile([128, 4 * C], BF16, tag="o", bufs=2, name=f"x{r}o{w}")
                for t4 in range(4):
                    tsl = slice(128 * t4, 128 * (t4 + 1))
                    for n2 in range(2):
                        nsl = slice(512 * n2, 512 * (n2 + 1))
                        ops_ = psp.tile([128, 512], F32, tag="s", bufs=SB, name=f"x{r}op{w}_{t4}_{n2}")
                        nc.tensor.matmul(
                            ops_, lhsT=yn[:, tsl], rhs=wpf[:, nsl],
                            start=True, stop=True, skip_group_check=True,
                        )
                        dst = osb[:, C * t4 + 512 * n2 : C * t4 + 512 * (n2 + 1)]
                        if n2 == 0 or not OSB_ON_ACT:
                            nc.vector.tensor_copy(out=dst, in_=ops_)
                        else:
                            nc.scalar.activation(out=dst, in_=ops_, func=mybir.ActivationFunctionType.Copy)
                    yield
                nc.scalar.dma_start(out=out[128 * w : 128 * (w + 1), :], in_=osb)

            def drain(gen, n=None):
                if gen is None:
                    return None
                try:
                    if n is None:
                        for _ in gen:
                            pass
                        return None
                    for _ in range(n):
                        next(gen)
                    return gen
                except StopIteration:
                    return None

            def anchor_stage1(r):
                asb = work.tile([128, 4 * C], BF16, tag="o", bufs=2, name=f"anch{r}")
                nc.vector.tensor_copy(out=asb[:, 0:512], in_=qrope_sb[:, 0:512])
                nc.vector.tensor_copy(out=asb[0:64, 512:1024], in_=krope_sb[:, 0:512])
                nc.vector.tensor_copy(out=asb[64:128, 512:768], in_=q1_sb[0:64, 0:256])
                for _vt in range(4):
                    nc.vector.tensor_copy(out=asb[64:128, 1024 + 64 * _vt : 1088 + 64 * _vt], in_=vaug_sb[64:128, _vt, 0:64])
                nc.scalar.dma_start(out=out[128 * (r % 4) : 128 * (r % 4 + 1), :], in_=asb)

            def anchor_attn(r, w, yps):
                asb = work.tile([128, 4 * C], BF16, tag="o", bufs=2, name=f"anch{r}_{w}")
                nc.vector.tensor_copy(out=asb[0:65, 0:512], in_=yps[0][:, 0, :])
                yield
                nc.vector.tensor_copy(out=asb[0:65, 512:1024], in_=yps[0][:, 1, :])
                yield
                nc.scalar.dma_start(out=out[128 * w : 128 * (w + 1), :], in_=asb)

            # ------- software-pipelined drive over (rep, window) -------
            if PARTIAL == 1:
                for r in range(repeat):
                    for w in range(NW):
                        if w == NW - 1 and r + 1 < repeat:
                            for cw in range(NW):
                                load_xt_chunk(cw)
                        drain(stage1_units(r, w))
                    anchor_stage1(r)
            else:
                first = stage1_units(0, 0)
                drain(first)
                epi_pend = None
                for r in range(repeat):
                    for w in range(NW):
                        if r + 1 < repeat:
                            # spread next rep's x^T reloads over windows 2-3
                            if w == 2:
                                load_xt_chunk(0)
                                load_xt_chunk(1)
                            elif w == 3:
                                load_xt_chunk(2)
                                load_xt_chunk(3)
                        if w + 1 < NW:
                            nxt = stage1_units(r, w + 1)
                        elif r + 1 < repeat:
                            nxt = stage1_units(r + 1, 0)
                        else:
                            nxt = None
                        yps = {}
                        for _ in attn_units(r, w, yps):
                            if EPI_FIRST:
                                epi_pend = drain(epi_pend, 2)
                                nxt = drain(nxt, 3)
                            else:
                                nxt = drain(nxt, 3)
                                epi_pend = drain(epi_pend, 1)
                        drain(nxt)
                        drain(epi_pend)
                        if PARTIAL == 2:
                            epi_pend = anchor_attn(r, w, yps)
                        else:
                            epi_pend = epilogue_units(r, w, yps)
                drain(epi_pend)
    nc.finalize()
    return nc


def _get_nc(repeat: int = 1) -> bass.Bass:
    key = ("nc", repeat)
    if key not in _CACHE:
        _CACHE[key] = _build_bass(repeat)
    return _CACHE[key]


def _make_in_maps(x, w_attn, w_proj):
    xT = np.ascontiguousarray(np.asarray(x, dtype=np.float32).reshape(T, C).T)  # [C, T]
    # [128, w, c, t'] pre-tiled so each chunk load is 8KB-contiguous/partition
    x2 = np.ascontiguousarray(
        xT.reshape(8, 128, NW, CW).transpose(1, 2, 0, 3).reshape(128, NW * 8 * CW)
    ).astype(ml_dtypes.bfloat16)
    wr = np.asarray(w_attn, dtype=np.float32).reshape(C, 4, 6, D).astype(ml_dtypes.bfloat16)
    wp = np.asarray(w_proj, dtype=np.float32).astype(ml_dtypes.bfloat16)
    in_maps = []
    for c in range(8):
        g = c // 2
        s = (2 * c) % 4
        wqkv_c = np.concatenate(
            [wr[:, g, s, :], wr[:, g, s + 1, :], wr[:, g, 4, :], wr[:, g, 5, :]], axis=1
        )  # [1024, 256]
        wqkv_c = np.ascontiguousarray(
            wqkv_c.reshape(8, 128, 256).transpose(1, 0, 2).reshape(128, 8 * 256)
        )
        wproj_c = np.ascontiguousarray(wp[128 * c : 128 * (c + 1), :])
        in_maps.append({"xt": x2, "wqkv": wqkv_c, "wproj": wproj_c})
    return in_maps


def _combine(results):
    acc = np.zeros((NW, 4, 128, C), np.float64)
    for r in results:
        # stored as [w*128+p, t4*C+c] -> logical t = 512w + 128*t4 + p
        acc += r["out"].reshape(NW, 128, 4, C).transpose(0, 2, 1, 3)
    return acc.astype(np.float32).reshape(1, T, C)


def run_for_test(inputs, trace=False):
    """Returns (output, exec_time_ns_or_None). Used by test.py."""
    nc = _get_nc()
    in_maps = _make_in_maps(**inputs)
    res = run_bass_kernel_spmd(nc, in_maps, core_ids=list(range(8)), trace=trace)
    return _combine(res.results), res.exec_time_ns


def kernel(x, w_attn, w_proj):
    out, _ = run_for_test({"x": x, "w_attn": w_attn, "w_proj": w_proj})
    return out
